# revision 1
# baseline (speedup 1.0000x reference)
"""GCN (3-layer + mean-pool + linear + softmax) on 8 Trainium2 NeuronCores.

Graph-parallel sharding: 8 contiguous node ranges; edges partitioned by
destination core, grouped by (128-node dst window, 25088-node src chunk),
padded to 128-slot blocks uniformly across cores (single SPMD program).
Per layer: y = xW on PE, y~ = dinv*y, AllGather into an fp32 table,
bulk dma_gather of y~[src] (int16 chunk-local indices, 256B rows),
one-hot(dst) matmul reduction into PSUM per window, epilogue
x' = relu(dinv*(agg + y~_self) + b). Pooling via one-hot(graph) matmuls
into per-core [64,65] partials; host sums partials and applies the final
64x10 linear + softmax. deg/dinv precomputed on host.
"""
import os
import sys
import numpy as np

sys.path.insert(0, os.path.dirname(os.path.abspath(__file__)))

N_NODES = 100000
N_GRAPHS = 256
IN_DIM = 128
F = 64
OUT_DIM = 10
C = 8
NODES_C = 12500
PADN = 12544          # 98 * 128
W = 98
NTOT = C * PADN       # 100352
CH = 25088            # int16 chunk size; 4 chunks cover NTOT
NCHUNK = 4
SUB = 32              # gather sub-op size in 128-slot blocks

_prog_cache = {}


def _roundup(x, m):
    return (x + m - 1) // m * m


# --------------------------------------------------------------------------
# wait-splitting workaround: this walrus build rejects >1 sem wait on one
# instruction ("Too many sync wait commands"); hoist extras onto injected
# same-engine InstEventSemaphore waits.
def _split_waits(nc, cap=1):
    import concourse.mybir as mybir
    uid = [0]
    n_fixed = 0
    for fn in nc.m.functions:
        for bb in fn.blocks:
            insts = bb.instructions
            new_list = []
            for inst in insts:
                si = inst.sync_info
                waits = list(si.on_wait) if si and si.on_wait else []
                if len(waits) > cap:
                    extra, keep = waits[:-cap], waits[-cap:]
                    for wv in extra:
                        uid[0] += 1
                        nop = mybir.InstEventSemaphore(name=f"waitfix_{uid[0]}")
                        nop.engine = inst.engine
                        nop.sync_info = mybir.SyncInfo(on_wait=[wv], on_update=[])
                        new_list.append(nop)
                    si.on_wait = keep
                    n_fixed += 1
                new_list.append(inst)
            if len(new_list) != len(insts):
                try:
                    bb.instructions = new_list
                except Exception:
                    insts.clear()
                    insts.extend(new_list)
    return n_fixed


def _build_program(op_ranges, win_of_block, blocks_total):
    import concourse.bacc as bacc
    import concourse.mybir as mybir
    import concourse.tile as tile

    f32 = mybir.dt.float32
    bf16 = mybir.dt.bfloat16
    i16 = mybir.dt.int16
    AF = mybir.ActivationFunctionType
    OP = mybir.AluOpType

    TOTB = blocks_total
    TOT_SLOTS = TOTB * 128

    first_of_win = {}
    last_of_win = {}
    for b, w in enumerate(win_of_block):
        if w not in first_of_win:
            first_of_win[w] = b
        last_of_win[w] = b

    nc = bacc.Bacc("TRN2", target_bir_lowering=False, debug=False, num_devices=C)

    xT_in = nc.declare_dram_parameter("xT", [IN_DIM, PADN], bf16, isOutput=False)
    W1_in = nc.declare_dram_parameter("W1", [IN_DIM, F], bf16, isOutput=False)
    W2_in = nc.declare_dram_parameter("W2", [F, F], bf16, isOutput=False)
    W3_in = nc.declare_dram_parameter("W3", [F, F], bf16, isOutput=False)
    ball_in = nc.declare_dram_parameter("ball", [128, 3 * F], f32, isOutput=False)
    iota128_in = nc.declare_dram_parameter("iota128", [128, 128], bf16, isOutput=False)
    iota64_in = nc.declare_dram_parameter("iota64", [128, F], f32, isOutput=False)
    ident_in = nc.declare_dram_parameter("ident", [128, 128], bf16, isOutput=False)
    ones_in = nc.declare_dram_parameter("onescol", [128, 1], f32, isOutput=False)
    dstl_in = nc.declare_dram_parameter("dstl", [128, TOTB], f32, isOutput=False)
    wsl_in = nc.declare_dram_parameter("wsl", [128, TOTB], f32, isOutput=False)
    idx_in = nc.declare_dram_parameter("idx16", [128, TOT_SLOTS // 16], i16, isOutput=False)
    dinv_in = nc.declare_dram_parameter("dinv", [128, W], f32, isOutput=False)
    bl_in = nc.declare_dram_parameter("batchloc", [128, W], f32, isOutput=False)
    pool_out = nc.declare_dram_parameter("pool_out", [F, F + 1], f32, isOutput=True)

    from contextlib import ExitStack
    stk = ExitStack()
    ag_sems = [stk.enter_context(nc.semaphore(f"ag_sem_{i}")) for i in range(3)]
    cc_sems = [stk.enter_context(nc.semaphore(f"cc_sem_{i}")) for i in range(3)]
    with tile.TileContext(nc, num_cores=C) as tc:
        tc.race_detector_enabled = False
        with (
            tc.tile_pool(name="persist", bufs=1) as pp,
            tc.tile_pool(name="sbuf", bufs=4) as sb,
            tc.tile_pool(name="msgp", bufs=6) as mp,
            tc.tile_pool(name="psum", bufs=2, space="PSUM") as ps,
            tc.tile_pool(name="poolps", bufs=1, space="PSUM") as ps1,
            tc.tile_pool(name="dram", bufs=1, space="DRAM") as dr,
        ):
            def load(name, shape, dt, src):
                t = pp.tile(shape, dt, name=name)
                nc.sync.dma_start(out=t[:], in_=src[:])
                return t

            xT_a = load("xT_a", [IN_DIM, PADN], bf16, xT_in)
            w1 = load("w1", [IN_DIM, F], bf16, W1_in)
            w2 = load("w2", [F, F], bf16, W2_in)
            w3 = load("w3", [F, F], bf16, W3_in)
            ball = load("ball", [128, 3 * F], f32, ball_in)
            iota128 = load("iota128", [128, 128], bf16, iota128_in)
            iota64 = load("iota64", [128, F], f32, iota64_in)
            ident = load("ident", [128, 128], bf16, ident_in)
            onescol = load("onescol", [128, 1], f32, ones_in)
            dstl = load("dstl", [128, TOTB], f32, dstl_in)
            wsl = load("wsl", [128, TOTB], f32, wsl_in)
            idx16 = load("idx16", [128, TOT_SLOTS // 16], i16, idx_in)
            dinv = load("dinv", [128, W], f32, dinv_in)
            batchloc = load("batchloc", [128, W], f32, bl_in)

            yt = pp.tile([128, W * F], f32, name="yt")
            acc = pp.tile([128, W * F], f32, name="acc")
            xTn = pp.tile([F, PADN], bf16, name="xTn")

            for layer in range(3):
                wmat = (w1, w2, w3)[layer]

                # phase A: y~ = dinv * (x @ W)
                for w in range(W):
                    psum_y = ps.tile([128, F], f32, name="psum_y", tag="psum_y", bufs=1)
                    lhsT = (xT_a if layer == 0 else xTn)[:, w * 128:(w + 1) * 128]
                    nc.tensor.matmul(psum_y[:], lhsT, wmat[:], start=True, stop=True)
                    nc.vector.tensor_scalar_mul(
                        yt[:, w * F:(w + 1) * F], psum_y[:], dinv[:, w:w + 1]
                    )

                ag_in = dr.tile([PADN, F], f32, name=f"ag_in_{layer}")
                table = dr.tile([NTOT, F], f32, name=f"table_{layer}")
                with tc.tile_critical():
                    nc.gpsimd.dma_start(
                        out=ag_in[:].rearrange("(p w) f -> p w f", p=128),
                        in_=yt[:].rearrange("p (w f) -> p w f", f=F),
                    ).then_inc(ag_sems[layer], 16)
                    nc.gpsimd.wait_ge(ag_sems[layer], 16)
                    nc.gpsimd.collective_compute(
                        "AllGather",
                        mybir.AluOpType.bypass,
                        replica_groups=[list(range(C))],
                        ins=[ag_in.opt()],
                        outs=[table.opt()],
                    ).then_inc(cc_sems[layer], 1)
                    nc.gpsimd.wait_ge(cc_sems[layer], 1)



                # phase B: bulk gathers of y~[src] (chunk-local int16 indices)
                msg_tiles = []
                for (s0, s1, q) in op_ranges:
                    nb = (s1 - s0) // 128
                    m = mp.tile([128, SUB, F], f32, name="msg", tag="msg")
                    nc.gpsimd.dma_gather(
                        out_ap=m[:, :nb, :],
                        in_ap=table[q * CH:(q + 1) * CH],
                        idxs_ap=idx16[:, s0 // 16:s1 // 16],
                        num_idxs=s1 - s0,
                        num_idxs_reg=s1 - s0,
                        elem_size=F,
                    )
                    msg_tiles.append((m, s0 // 128, nb))

                # phase C: one-hot reduction + inline per-window epilogue
                op_i = 0
                psums = {}
                cur = None
                for b in range(TOTB):
                    w = win_of_block[b]
                    m, b0, nb = msg_tiles[op_i]
                    if b >= b0 + nb:
                        op_i += 1
                        m, b0, nb = msg_tiles[op_i]
                    if b == first_of_win[w]:
                        psums[w] = ps.tile([128, F], f32, name="psum_agg",
                                           tag="psum_agg", bufs=2)
                    st = sb.tile([128, 128], bf16, name="st", tag="st")
                    nc.vector.tensor_scalar(
                        st[:], iota128[:], dstl[:, b:b + 1], None, OP.is_equal
                    )
                    mb = sb.tile([128, F], bf16, name="mb", tag="mb")
                    if b % 2 == 0:
                        nc.vector.tensor_scalar_mul(
                            mb[:], m[:, b - b0, :], wsl[:, b:b + 1]
                        )
                    else:
                        nc.scalar.activation(
                            mb[:], m[:, b - b0, :], AF.Copy, scale=wsl[:, b:b + 1]
                        )
                    nc.tensor.matmul(
                        psums[w][:], st[:], mb[:],
                        start=(b == first_of_win[w]),
                        stop=(b == last_of_win[w]),
                    )
                    if b == last_of_win[w]:
                        _emit_epilogue(nc, tc, sb, ps, layer, w, psums.pop(w),
                                       yt, acc, dinv, ball, ident, xTn,
                                       f32, bf16, AF, OP)

                # windows with no incident edges at all (no blocks)
                for w in range(W):
                    if w not in first_of_win:
                        _emit_epilogue(nc, tc, sb, ps, layer, w, None,
                                       yt, acc, dinv, ball, ident, xTn,
                                       f32, bf16, AF, OP)

            # pooling: one-hot(graph-id) matmuls, accumulated over windows
            psum_sums = ps1.tile([F, F], f32, name="psum_sums")
            psum_cnt = ps1.tile([F, 1], f32, name="psum_cnt")
            for w in range(W):
                sg = sb.tile([128, F], f32, name="sg", tag="sg")
                nc.vector.tensor_scalar(
                    sg[:], iota64[:], batchloc[:, w:w + 1], None, OP.is_equal
                )
                nc.tensor.matmul(
                    psum_sums[:], sg[:], acc[:, w * F:(w + 1) * F],
                    start=(w == 0), stop=(w == W - 1), skip_group_check=True,
                )
                nc.tensor.matmul(
                    psum_cnt[:], sg[:], onescol[:],
                    start=(w == 0), stop=(w == W - 1), skip_group_check=True,
                )
            outt = sb.tile([F, F + 1], f32, name="outt", tag="outt")
            nc.vector.tensor_copy(outt[:, :F], psum_sums[:])
            nc.vector.tensor_copy(outt[:, F:F + 1], psum_cnt[:])
            nc.sync.dma_start(out=pool_out[:], in_=outt[:])

    stk.close()
    nc.compile()
    _split_waits(nc)
    return nc


def _emit_epilogue(nc, tc, sb, ps, layer, w, psum_agg, yt, acc, dinv, ball,
                   ident, xTn, f32, bf16, AF, OP):
    """x' = relu(dinv * (agg + y~_self) + b); acc update; transpose for next."""
    ytw = yt[:, w * F:(w + 1) * F]
    xt = sb.tile([128, F], f32, name="xt", tag="xt")
    if psum_agg is not None:
        nc.vector.tensor_add(xt[:], psum_agg[:], ytw)
    else:
        nc.vector.tensor_copy(xt[:], ytw)
    nc.vector.tensor_scalar_mul(xt[:], xt[:], dinv[:, w:w + 1])
    nc.vector.tensor_add(xt[:], xt[:], ball[:, layer * F:(layer + 1) * F])
    xr = sb.tile([128, F], f32, name="xr", tag="xr")
    nc.scalar.activation(xr[:], xt[:], AF.Relu)
    accw = acc[:, w * F:(w + 1) * F]
    if layer == 0:
        nc.vector.tensor_copy(accw, xr[:])
    else:
        nc.vector.tensor_add(accw, accw, xr[:])
    if layer < 2:
        xb = sb.tile([128, F], bf16, name="xb", tag="xb")
        nc.vector.tensor_copy(xb[:], xr[:])
        ptr = ps.tile([F, 128], bf16, name="ptr", tag="ptr", bufs=1)
        nc.tensor.transpose(ptr[:], xb[:], ident[:])
        nc.scalar.activation(xTn[:, w * 128:(w + 1) * 128], ptr[:], AF.Copy)


# --------------------------------------------------------------------------
def _host_prep(x, edge_weight, edge_index, batch):
    src = np.asarray(edge_index[0], dtype=np.int64)
    dst = np.asarray(edge_index[1], dtype=np.int64)
    w_abs = np.abs(np.asarray(edge_weight, dtype=np.float32))
    batch = np.asarray(batch, dtype=np.int64)
    x = np.asarray(x, dtype=np.float32)

    core = dst // NODES_C
    dstl_full = dst - core * NODES_C
    win = dstl_full >> 7
    lane = dstl_full & 127
    srcl = src % NODES_C
    srcp = (src // NODES_C) * PADN + (srcl & 127) * W + (srcl >> 7)
    q = srcp // CH
    qloc = (srcp - q * CH).astype(np.int16)

    key = (core * W + win) * NCHUNK + q
    order = np.argsort(key, kind="stable")
    key_s = key[order]
    core_s = core[order]
    qloc_s = qloc[order]
    w_s = w_abs[order]
    lane_s = lane[order].astype(np.float32)

    counts = np.bincount(key, minlength=C * W * NCHUNK).reshape(C, W * NCHUNK)
    P_wq = np.array([_roundup(m, 128) for m in counts.max(axis=0)])  # [W*4]
    base = np.concatenate([[0], np.cumsum(P_wq)])                    # [W*4+1]
    TOT_SLOTS = int(base[-1])
    TOTB = TOT_SLOTS // 128

    grp_start = np.concatenate([[0], np.cumsum(np.bincount(key_s, minlength=C * W * NCHUNK))])
    rank = np.arange(len(key_s)) - grp_start[key_s]
    wq = key_s % (W * NCHUNK)
    slotpos = base[wq] + rank

    qloc_pad = np.zeros((C, TOT_SLOTS), dtype=np.int16)
    w_pad = np.zeros((C, TOT_SLOTS), dtype=np.float32)
    lane_pad = np.zeros((C, TOT_SLOTS), dtype=np.float32)
    qloc_pad[core_s, slotpos] = qloc_s
    w_pad[core_s, slotpos] = w_s
    lane_pad[core_s, slotpos] = lane_s

    # block tables
    blocks_per_wq = (P_wq // 128).astype(np.int64)
    win_of_block = np.repeat(np.arange(W * NCHUNK) // NCHUNK, blocks_per_wq).tolist()
    q_of_block = np.repeat(np.arange(W * NCHUNK) % NCHUNK, blocks_per_wq)

    # gather sub-op ranges: contiguous runs of blocks with the same chunk,
    # split to <= SUB blocks (slot offsets, multiples of 128)
    op_ranges = []
    b = 0
    TOTB_l = len(win_of_block)
    while b < TOTB_l:
        b2 = b
        while b2 < TOTB_l and q_of_block[b2] == q_of_block[b] and b2 - b < SUB:
            b2 += 1
        op_ranges.append((b * 128, b2 * 128, int(q_of_block[b])))
        b = b2
    # pad qloc so every slot of an op shares the op's chunk (pads use row 0 of
    # that chunk; w=0 kills their contribution). qloc_pad already 0 -> row 0. ok

    # per-lane views [C, 128, TOTB]
    dstl_lane = lane_pad.reshape(C, TOTB, 128).transpose(0, 2, 1)
    wsl_lane = w_pad.reshape(C, TOTB, 128).transpose(0, 2, 1)

    # idx16 wrapped per sub-op: idx k of op -> partition k%16, col (s0+k)//16,
    # replicated across the 8 groups of 16 partitions
    idx_arr = np.zeros((C, 16, TOT_SLOTS // 16), dtype=np.int16)
    for (s0, s1, _q) in op_ranges:
        seg = qloc_pad[:, s0:s1].reshape(C, (s1 - s0) // 16, 16)
        idx_arr[:, :, s0 // 16:s1 // 16] = seg.transpose(0, 2, 1)
    idx_full = np.tile(idx_arr, (1, 8, 1))

    # degrees -> dinv (host: scalar per node)
    deg = np.bincount(dst, weights=w_abs.astype(np.float64), minlength=N_NODES)
    deg = deg + 1.0
    dinv_full = (1.0 / np.sqrt(deg)).astype(np.float32)
    dinv_lane = np.ones((C, 128, W), dtype=np.float32)
    nodes = np.arange(NODES_C)
    for c in range(C):
        dv = dinv_full[c * NODES_C:(c + 1) * NODES_C]
        dinv_lane[c, nodes & 127, nodes >> 7] = dv

    # batch local ids
    gmin = np.zeros(C, dtype=np.int64)
    bl_lane = np.full((C, 128, W), 63.0, dtype=np.float32)
    for c in range(C):
        bseg = batch[c * NODES_C:(c + 1) * NODES_C]
        gmin[c] = bseg[0]
        rng = int(bseg[-1] - bseg[0])
        assert rng <= 62, f"graph range {rng} too large for pooling layout"
        bl_lane[c, nodes & 127, nodes >> 7] = (bseg - gmin[c]).astype(np.float32)

    # xT per core, bf16, padded
    import jax.numpy as jnp
    xT = np.zeros((C, IN_DIM, PADN), dtype=np.float32)
    for c in range(C):
        xT[c, :, :NODES_C] = x[c * NODES_C:(c + 1) * NODES_C].T
    xT_bf = np.asarray(jnp.asarray(xT, jnp.bfloat16))

    return dict(
        op_ranges=op_ranges, win_of_block=win_of_block, TOTB=TOTB_l,
        dstl_lane=dstl_lane, wsl_lane=wsl_lane, idx_full=idx_full,
        dinv_lane=dinv_lane, bl_lane=bl_lane, xT_bf=xT_bf, gmin=gmin,
    )


def kernel(x, edge_weight, W1, b1, W2, b2, W3, b3, Wl, bl, edge_index, batch):
    from concourse.bass_utils import run_bass_kernel_spmd
    import jax.numpy as jnp

    prep = _host_prep(x, edge_weight, edge_index, batch)

    cache_key = (prep["TOTB"], tuple(prep["op_ranges"][:3]))
    if cache_key not in _prog_cache:
        _prog_cache[cache_key] = _build_program(
            prep["op_ranges"], prep["win_of_block"], prep["TOTB"]
        )
    nc = _prog_cache[cache_key]

    bf = lambda a: np.asarray(jnp.asarray(np.asarray(a, np.float32), jnp.bfloat16))
    W1b, W2b, W3b = bf(W1), bf(W2), bf(W3)
    ball = np.zeros((128, 3 * F), dtype=np.float32)
    ball[:, 0:F] = np.asarray(b1, np.float32)[None, :]
    ball[:, F:2 * F] = np.asarray(b2, np.float32)[None, :]
    ball[:, 2 * F:3 * F] = np.asarray(b3, np.float32)[None, :]
    iota128 = np.asarray(jnp.asarray(
        np.tile(np.arange(128, dtype=np.float32)[None, :], (128, 1)), jnp.bfloat16))
    iota64 = np.tile(np.arange(F, dtype=np.float32)[None, :], (128, 1))
    ident = np.asarray(jnp.asarray(np.eye(128, dtype=np.float32), jnp.bfloat16))
    onescol = np.ones((128, 1), dtype=np.float32)

    in_maps = []
    for c in range(C):
        in_maps.append({
            "xT": prep["xT_bf"][c],
            "W1": W1b, "W2": W2b, "W3": W3b, "ball": ball,
            "iota128": iota128, "iota64": iota64, "ident": ident,
            "onescol": onescol,
            "dstl": prep["dstl_lane"][c], "wsl": prep["wsl_lane"][c],
            "idx16": prep["idx_full"][c],
            "dinv": prep["dinv_lane"][c], "batchloc": prep["bl_lane"][c],
        })

    res = run_bass_kernel_spmd(nc, in_maps, core_ids=list(range(C)))

    sums = np.zeros((N_GRAPHS, F), dtype=np.float64)
    cnts = np.zeros(N_GRAPHS, dtype=np.float64)
    for c in range(C):
        out = res.results[c]["pool_out"]
        g0 = int(prep["gmin"][c])
        for r in range(63):
            g = g0 + r
            if g < N_GRAPHS:
                sums[g] += out[r, :F]
                cnts[g] += out[r, F]
    pooled = (sums / 3.0) / np.maximum(cnts, 1.0)[:, None]
    logits = pooled @ np.asarray(Wl, np.float64) + np.asarray(bl, np.float64)
    z = logits - logits.max(axis=1, keepdims=True)
    ez = np.exp(z)
    return (ez / ez.sum(axis=1, keepdims=True)).astype(np.float32)

